# revision 4
# baseline (speedup 1.0000x reference)
# Ragged-sequence cross-attention on 8 TRN2 NeuronCores.
#
# reference: scores = einsum('tbh,sbh->bts', dec, enc); mask s >= lens[b];
#            weights = softmax(scores, axis=s); context = einsum('bts,sbh->bth', w, enc)
# Shapes: S=4096, B=8, T=512, H=400.
#
# Sharding: pure data-parallel, core b owns batch element b (B == 8 == cores).
#
# Device kernel per core (single pass over scores, no collectives):
#   - scores[t, s] tiles ([128 t, 512 s] PSUM) via fp32r matmuls contracting H
#     in 4 chunks; the ragged mask rides as an extra contraction row
#     (decT row 400 = 1.0, encT row 400 = 0 / -1e9).
#   - per-tile local softmax: rowmax (DVE, negated), exp(score - max) on ACT
#     with accum_out giving the local denominator partial -> fp16 tiles.
#   - once a 128-t row-block's 8 tiles are done: factors = exp(m_si - m_fin)
#     (tiny ACT op) rescale all 8 tiles to the row-global max scale (DVE).
#   - rescaled fp16 weight tiles are DMA'd out (host divides by the summed
#     denominator) and transposed on the PE (128x128 identity matmuls) to
#     become lhsT for the context matmul: ctx_un[t,h] = sum_s wT[s,t] enc[s,h].
#   - host: d = sum_si dn_si * exp(m_si - m_fin); weights = w_un / d;
#     context = ctx_un / d.

import sys

sys.path.insert(0, "/opt/trn_rl_repo")

from contextlib import ExitStack

import numpy as np

import concourse.bass as bass
import concourse.tile as tile
from concourse import bacc, mybir
from concourse.bass_utils import run_bass_kernel_spmd
from concourse.masks import make_identity

S, B, T, H = 4096, 8, 512, 400
N_CORES = 8
NEG = -1.0e9

_f32 = mybir.dt.float32
_f32r = mybir.dt.float32r
_f16 = mybir.dt.float16

KCH = [(0, 128), (128, 128), (256, 128), (384, 17)]  # H chunks + mask row 400
NT = T // 128  # 4 row blocks of t
NS = S // 512  # 8 col blocks of s (scores layout)
NS2 = S // 128  # 32 s blocks of 128 (ctx contraction)


def _build_program():
    nc = bacc.Bacc("TRN2", target_bir_lowering=False, debug=False, num_devices=N_CORES)

    encT = nc.dram_tensor("encT", [H + 1, S], _f32r, kind="ExternalInput").ap()
    decT = nc.dram_tensor("decT", [H + 1, T], _f32r, kind="ExternalInput").ap()
    encN = nc.dram_tensor("encN", [S, H], _f16, kind="ExternalInput").ap()

    w_un = nc.dram_tensor("w_un", [T, S], _f16, kind="ExternalOutput").ap()
    ctx_un = nc.dram_tensor("ctx_un", [T, H], _f32, kind="ExternalOutput").ap()
    mx_d = nc.dram_tensor("mx", [T], _f32, kind="ExternalOutput").ap()  # -m_fin
    mx8_d = nc.dram_tensor("mx8", [T, NS], _f32, kind="ExternalOutput").ap()  # -m_si
    dn8_d = nc.dram_tensor("dn8", [T, NS], _f32, kind="ExternalOutput").ap()

    with ExitStack() as ctx:
        tc = ctx.enter_context(tile.TileContext(nc))
        const = ctx.enter_context(tc.tile_pool(name="const", bufs=1))
        stat = ctx.enter_context(tc.tile_pool(name="stat", bufs=1))
        ex_pool = ctx.enter_context(tc.tile_pool(name="ex", bufs=18))
        wt_pool = ctx.enter_context(tc.tile_pool(name="wt", bufs=9))
        out_pool = ctx.enter_context(tc.tile_pool(name="outs", bufs=2))
        ps1 = ctx.enter_context(tc.tile_pool(name="ps1", bufs=3, space="PSUM"))
        pstr = ctx.enter_context(tc.tile_pool(name="pstr", bufs=2, space="PSUM"))
        psc = ctx.enter_context(tc.tile_pool(name="psc", bufs=2, space="PSUM"))

        # ---- resident inputs ----
        et = []
        for k, (k0, kn) in enumerate(KCH):
            t_ = const.tile([kn, S], _f32r, tag=f"et{k}")
            nc.sync.dma_start(t_[:], encT[k0 : k0 + kn, :])
            et.append(t_)
        dt_ = []
        for k, (k0, kn) in enumerate(KCH):
            t_ = const.tile([kn, T], _f32r, tag=f"dt{k}")
            nc.sync.dma_start(t_[:], decT[k0 : k0 + kn, :])
            dt_.append(t_)
        en = []
        for i in range(NS2):
            t_ = const.tile([128, H], _f16, tag=f"en{i}")
            nc.sync.dma_start(t_[:], encN[i * 128 : (i + 1) * 128, :])
            en.append(t_)
        ident = const.tile([128, 128], _f16, tag="ident")
        make_identity(nc, ident[:])

        ex_tiles = [[None] * NS for _ in range(NT)]
        stats = [None] * NT  # (nmxs, dns, fac)

        def phase_scores(tj):
            nmxs = stat.tile([128, NS], _f32, tag=f"nmxs{tj}")
            dns = stat.tile([128, NS], _f32, tag=f"dns{tj}")
            stats[tj] = (nmxs, dns)
            for si in range(NS):
                ps = ps1.tile([128, 512], _f32, tag="ps1")
                for k, (k0, kn) in enumerate(KCH):
                    nc.tensor.matmul(
                        ps[:],
                        dt_[k][:, tj * 128 : (tj + 1) * 128],
                        et[k][:, si * 512 : (si + 1) * 512],
                        start=(k == 0),
                        stop=(k == len(KCH) - 1),
                    )
                nc.vector.reduce_max(
                    out=nmxs[:, si : si + 1],
                    in_=ps[:],
                    axis=mybir.AxisListType.X,
                    negate=True,
                )
                ex = ex_pool.tile([128, 512], _f16, tag="ex")
                nc.scalar.activation(
                    ex[:],
                    ps[:],
                    mybir.ActivationFunctionType.Exp,
                    bias=nmxs[:, si : si + 1],
                    scale=1.0,
                    accum_out=dns[:, si : si + 1],
                )
                ex_tiles[tj][si] = ex

        def phase_stats(tj):
            nmxs, dns = stats[tj]
            nmf = stat.tile([128, 1], _f32, tag=f"nmf{tj}")  # -m_fin
            fac = stat.tile([128, NS], _f32, tag=f"fac{tj}")
            nc.vector.tensor_reduce(
                out=nmf[:], in_=nmxs[:], axis=mybir.AxisListType.X, op=mybir.AluOpType.min
            )
            # fac = exp(m_si - m_fin) = exp(-nmxs + nmf)
            nc.scalar.activation(
                fac[:],
                nmxs[:],
                mybir.ActivationFunctionType.Exp,
                bias=nmf[:],
                scale=-1.0,
            )
            stats[tj] = (nmxs, dns, fac)
            row = slice(tj * 128, (tj + 1) * 128)
            nc.sync.dma_start(mx_d[row], nmf[:])
            nc.sync.dma_start(mx8_d[row, :], nmxs[:])
            nc.sync.dma_start(dn8_d[row, :], dns[:])

        def phase_tail(tj):
            _, _, fac = stats[tj]
            row = slice(tj * 128, (tj + 1) * 128)
            wts = []
            for si in range(NS):
                ex = ex_tiles[tj][si]
                nc.vector.tensor_scalar_mul(ex[:], ex[:], fac[:, si : si + 1])
                nc.sync.dma_start(w_un[row, si * 512 : (si + 1) * 512], ex[:])
                pt = pstr.tile([128, 512], _f16, tag="pstr")
                for q in range(4):
                    nc.tensor.transpose(
                        pt[:, q * 128 : (q + 1) * 128],
                        ex[:, q * 128 : (q + 1) * 128],
                        ident[:],
                    )
                wt = wt_pool.tile([128, 512], _f16, tag="wt")
                nc.vector.tensor_copy(wt[:], pt[:])
                wts.append(wt)
                ex_tiles[tj][si] = None
            cps = psc.tile([128, H], _f32, tag="cps")
            for si in range(NS):
                for q in range(4):
                    i = 4 * si + q
                    nc.tensor.matmul(
                        cps[:],
                        wts[si][:, q * 128 : (q + 1) * 128],
                        en[i][:],
                        start=(i == 0),
                        stop=(i == NS2 - 1),
                    )
            csb = out_pool.tile([128, H], _f32, tag="csb")
            nc.vector.tensor_copy(csb[:], cps[:])
            nc.sync.dma_start(ctx_un[row, :], csb[:])

        # software-pipelined trace order: scores(tj+1) fills the PE while
        # tj's exp/rescale runs on ACT/DVE.
        phase_scores(0)
        phase_stats(0)
        for tj in range(NT):
            if tj + 1 < NT:
                phase_scores(tj + 1)
                phase_stats(tj + 1)
            phase_tail(tj)

    nc.compile()
    return nc


def _make_in_maps(encoder_outputs, decoder_output, lens):
    enc = np.asarray(encoder_outputs, dtype=np.float32)
    dec = np.asarray(decoder_output, dtype=np.float32)
    lens = np.asarray(lens).astype(np.int64)
    in_maps = []
    s_idx = np.arange(S)
    for b in range(N_CORES):
        enc_b = enc[:, b, :]  # [S, H]
        encT = np.empty((H + 1, S), dtype=np.float32)
        encT[:H] = enc_b.T
        encT[H] = np.where(s_idx < lens[b], 0.0, NEG)
        decT = np.empty((H + 1, T), dtype=np.float32)
        decT[:H] = dec[:, b, :].T
        decT[H] = 1.0
        in_maps.append(
            {
                "encT": encT,
                "decT": decT,
                "encN": enc_b.astype(np.float16),
            }
        )
    return in_maps


def _combine(results):
    context = np.empty((B, T, H), dtype=np.float32)
    weights = np.empty((B, T, S), dtype=np.float32)
    for b in range(B):
        r = results[b]
        m_fin = -r["mx"].astype(np.float64)  # [T]
        m_si = -r["mx8"].astype(np.float64)  # [T, NS]
        d = (r["dn8"].astype(np.float64) * np.exp(m_si - m_fin[:, None])).sum(axis=1)
        inv = (1.0 / d)[:, None]
        weights[b] = r["w_un"].astype(np.float64) * inv
        context[b] = r["ctx_un"].astype(np.float64) * inv
    return context, weights


_PROGRAM_CACHE = {}


def _get_program():
    if "nc" not in _PROGRAM_CACHE:
        _PROGRAM_CACHE["nc"] = _build_program()
    return _PROGRAM_CACHE["nc"]


def kernel(encoder_outputs, decoder_output, lens, _trace=False, _trace_kwargs=None):
    nc = _get_program()
    in_maps = _make_in_maps(encoder_outputs, decoder_output, lens)
    res = run_bass_kernel_spmd(
        nc,
        in_maps,
        core_ids=list(range(N_CORES)),
        trace=_trace,
        **(_trace_kwargs or {}),
    )
    context, weights = _combine(res.results)
    if _trace:
        return (context, weights), res
    return context, weights


if __name__ == "__main__":
    rng = np.random.default_rng(0)
    enc = rng.standard_normal((S, B, H), dtype=np.float32)
    dec = rng.standard_normal((T, B, H), dtype=np.float32)
    lens = rng.integers(1, S + 1, size=(B,)).astype(np.int32)
    lens[0] = S
    ctx_out, w_out = kernel(enc, dec, lens)
    print("context", ctx_out.shape, "weights", w_out.shape)


# revision 6
# speedup vs baseline: 1.1063x; 1.1063x over previous
# Ragged-sequence cross-attention on 8 TRN2 NeuronCores.
#
# reference: scores = einsum('tbh,sbh->bts', dec, enc); mask s >= lens[b];
#            weights = softmax(scores, axis=s); context = einsum('bts,sbh->bth', w, enc)
# Shapes: S=4096, B=8, T=512, H=400.
#
# Sharding: pure data-parallel, core b owns batch element b (B == 8 == cores).
#
# Device kernel per core (single pass over scores, no collectives):
#   - scores[t, s] tiles ([128 t, 512 s] PSUM) via fp32r matmuls contracting H
#     in 4 chunks; the ragged mask rides as an extra contraction row
#     (decT row 400 = 1.0, encT row 400 = 0 / -1e9).
#   - per-tile local softmax: rowmax (DVE, negated), exp(score - max) on ACT
#     with accum_out giving the local denominator partial -> fp16 tiles.
#   - once a 128-t row-block's 8 tiles are done: factors = exp(m_si - m_fin)
#     (tiny ACT op) rescale all 8 tiles to the row-global max scale (DVE).
#   - rescaled fp16 weight tiles are DMA'd out (host divides by the summed
#     denominator) and transposed on the PE (128x128 identity matmuls) to
#     become lhsT for the context matmul: ctx_un[t,h] = sum_s wT[s,t] enc[s,h].
#   - host: d = sum_si dn_si * exp(m_si - m_fin); weights = w_un / d;
#     context = ctx_un / d.

import sys

sys.path.insert(0, "/opt/trn_rl_repo")

from contextlib import ExitStack

import numpy as np

import concourse.bass as bass
import concourse.tile as tile
from concourse import bacc, mybir
from concourse.bass_utils import run_bass_kernel_spmd
from concourse.masks import make_identity

S, B, T, H = 4096, 8, 512, 400
N_CORES = 8
NEG = -1.0e9

_f32 = mybir.dt.float32
_f32r = mybir.dt.float32r
_f16 = mybir.dt.float16

KCH = [(0, 128), (128, 128), (256, 128), (384, 17)]  # H chunks + mask row 400
NT = T // 128  # 4 row blocks of t
NS = S // 512  # 8 col blocks of s (scores layout)
NS2 = S // 128  # 32 s blocks of 128 (ctx contraction)


def _build_program():
    nc = bacc.Bacc("TRN2", target_bir_lowering=False, debug=False, num_devices=N_CORES)

    encT = nc.dram_tensor("encT", [H + 1, S], _f32r, kind="ExternalInput").ap()
    decT = nc.dram_tensor("decT", [H + 1, T], _f32r, kind="ExternalInput").ap()
    encN = nc.dram_tensor("encN", [128, NS2 * H], _f16, kind="ExternalInput").ap()

    w_un = nc.dram_tensor("w_un", [T, S], _f16, kind="ExternalOutput").ap()
    ctx_un = nc.dram_tensor("ctx_un", [T, H], _f32, kind="ExternalOutput").ap()
    mx_d = nc.dram_tensor("mx", [T], _f32, kind="ExternalOutput").ap()  # -m_fin
    mx8_d = nc.dram_tensor("mx8", [T, NS], _f32, kind="ExternalOutput").ap()  # -m_si
    dn8_d = nc.dram_tensor("dn8", [T, NS], _f32, kind="ExternalOutput").ap()

    with ExitStack() as ctx:
        tc = ctx.enter_context(tile.TileContext(nc))
        const = ctx.enter_context(tc.tile_pool(name="const", bufs=1))
        stat = ctx.enter_context(tc.tile_pool(name="stat", bufs=1))
        ex_pool = ctx.enter_context(tc.tile_pool(name="ex", bufs=3))
        wt_pool = ctx.enter_context(tc.tile_pool(name="wt", bufs=9))
        out_pool = ctx.enter_context(tc.tile_pool(name="outs", bufs=2))
        ps1 = ctx.enter_context(tc.tile_pool(name="ps1", bufs=4, space="PSUM"))
        pstr = ctx.enter_context(tc.tile_pool(name="pstr", bufs=2, space="PSUM"))
        psc = ctx.enter_context(tc.tile_pool(name="psc", bufs=2, space="PSUM"))

        # ---- resident inputs (fine-grained so the first matmuls start early) ----
        dt_ = []
        for k, (k0, kn) in enumerate(KCH):
            t_ = const.tile([kn, T], _f32r, tag=f"dt{k}")
            nc.gpsimd.dma_start(t_[:], decT[k0 : k0 + kn, :])
            dt_.append(t_)
        et = [[None] * NS for _ in KCH]
        for si in range(NS):
            for k, (k0, kn) in enumerate(KCH):
                t_ = const.tile([kn, 512], _f32r, tag=f"et{k}_{si}")
                nc.gpsimd.dma_start(
                    t_[:], encT[k0 : k0 + kn, si * 512 : (si + 1) * 512]
                )
                et[k][si] = t_
        enN = const.tile([128, NS2 * H], _f16, tag="enN")
        nc.gpsimd.dma_start(enN[:], encN[:])
        en = [enN[:, i * H : (i + 1) * H] for i in range(NS2)]
        ident = const.tile([128, 128], _f16, tag="ident")
        make_identity(nc, ident[:])

        ex_tiles = [None] * NT
        stats = [None] * NT  # (nmxs, dns, fac)

        def phase_scores(tj):
            ex_tj = ex_pool.tile([128, S], _f16, tag="ex")
            ex_tiles[tj] = ex_tj
            nmxs = stat.tile([128, NS], _f32, tag=f"nmxs{tj}")
            dns = stat.tile([128, NS], _f32, tag=f"dns{tj}")
            stats[tj] = (nmxs, dns)
            for si in range(NS):
                ps = ps1.tile([128, 512], _f32, tag="ps1")
                for k, (k0, kn) in enumerate(KCH):
                    nc.tensor.matmul(
                        ps[:],
                        dt_[k][:, tj * 128 : (tj + 1) * 128],
                        et[k][si][:],
                        start=(k == 0),
                        stop=(k == len(KCH) - 1),
                    )
                nc.vector.reduce_max(
                    out=nmxs[:, si : si + 1],
                    in_=ps[:],
                    axis=mybir.AxisListType.X,
                    negate=True,
                )
                nc.scalar.activation(
                    ex_tj[:, si * 512 : (si + 1) * 512],
                    ps[:],
                    mybir.ActivationFunctionType.Exp,
                    bias=nmxs[:, si : si + 1],
                    scale=1.0,
                    accum_out=dns[:, si : si + 1],
                )

        def phase_stats(tj):
            nmxs, dns = stats[tj]
            nmf = stat.tile([128, 1], _f32, tag=f"nmf{tj}")  # -m_fin
            fac = stat.tile([128, NS], _f32, tag=f"fac{tj}")
            nc.vector.tensor_reduce(
                out=nmf[:], in_=nmxs[:], axis=mybir.AxisListType.X, op=mybir.AluOpType.min
            )
            # fac = exp(m_si - m_fin) = exp(-nmxs + nmf)
            nc.scalar.activation(
                fac[:],
                nmxs[:],
                mybir.ActivationFunctionType.Exp,
                bias=nmf[:],
                scale=-1.0,
            )
            stats[tj] = (nmxs, dns, fac)
            row = slice(tj * 128, (tj + 1) * 128)
            nc.sync.dma_start(mx_d[row], nmf[:])
            nc.sync.dma_start(mx8_d[row, :], nmxs[:])
            nc.sync.dma_start(dn8_d[row, :], dns[:])

        def phase_tail(tj):
            _, _, fac = stats[tj]
            row = slice(tj * 128, (tj + 1) * 128)
            ex_tj = ex_tiles[tj]
            wts = []
            for si in range(NS):
                exs = ex_tj[:, si * 512 : (si + 1) * 512]
                nc.vector.tensor_scalar_mul(exs, exs, fac[:, si : si + 1])
                pt = pstr.tile([128, 512], _f16, tag="pstr")
                for q in range(4):
                    nc.tensor.transpose(
                        pt[:, q * 128 : (q + 1) * 128],
                        exs[:, q * 128 : (q + 1) * 128],
                        ident[:],
                    )
                wt = wt_pool.tile([128, 512], _f16, tag="wt")
                nc.vector.tensor_copy(wt[:], pt[:])
                wts.append(wt)
            nc.sync.dma_start(w_un[row, :], ex_tj[:])
            ex_tiles[tj] = None
            cps = psc.tile([128, H], _f32, tag="cps")
            for si in range(NS):
                for q in range(4):
                    i = 4 * si + q
                    nc.tensor.matmul(
                        cps[:],
                        wts[si][:, q * 128 : (q + 1) * 128],
                        en[i],
                        start=(i == 0),
                        stop=(i == NS2 - 1),
                    )
            csb = out_pool.tile([128, H], _f32, tag="csb")
            nc.vector.tensor_copy(csb[:], cps[:])
            nc.sync.dma_start(ctx_un[row, :], csb[:])

        # software-pipelined trace order: scores(tj+1) fills the PE while
        # tj's exp/rescale runs on ACT/DVE.
        phase_scores(0)
        phase_stats(0)
        for tj in range(NT):
            if tj + 1 < NT:
                phase_scores(tj + 1)
                phase_stats(tj + 1)
            phase_tail(tj)

    nc.compile()
    return nc


def _make_in_maps(encoder_outputs, decoder_output, lens):
    enc = np.asarray(encoder_outputs, dtype=np.float32)
    dec = np.asarray(decoder_output, dtype=np.float32)
    lens = np.asarray(lens).astype(np.int64)
    in_maps = []
    s_idx = np.arange(S)
    for b in range(N_CORES):
        enc_b = enc[:, b, :]  # [S, H]
        encT = np.empty((H + 1, S), dtype=np.float32)
        encT[:H] = enc_b.T
        encT[H] = np.where(s_idx < lens[b], 0.0, NEG)
        decT = np.empty((H + 1, T), dtype=np.float32)
        decT[:H] = dec[:, b, :].T
        decT[H] = 1.0
        in_maps.append(
            {
                "encT": encT,
                "decT": decT,
                "encN": np.ascontiguousarray(
                    enc_b.reshape(NS2, 128, H).transpose(1, 0, 2).reshape(128, NS2 * H)
                ).astype(np.float16),
            }
        )
    return in_maps


def _combine(results):
    context = np.empty((B, T, H), dtype=np.float32)
    weights = np.empty((B, T, S), dtype=np.float32)
    for b in range(B):
        r = results[b]
        m_fin = -r["mx"].astype(np.float64)  # [T]
        m_si = -r["mx8"].astype(np.float64)  # [T, NS]
        d = (r["dn8"].astype(np.float64) * np.exp(m_si - m_fin[:, None])).sum(axis=1)
        inv = (1.0 / d)[:, None]
        weights[b] = r["w_un"].astype(np.float64) * inv
        context[b] = r["ctx_un"].astype(np.float64) * inv
    return context, weights


_PROGRAM_CACHE = {}


def _get_program():
    if "nc" not in _PROGRAM_CACHE:
        _PROGRAM_CACHE["nc"] = _build_program()
    return _PROGRAM_CACHE["nc"]


def kernel(encoder_outputs, decoder_output, lens, _trace=False, _trace_kwargs=None):
    nc = _get_program()
    in_maps = _make_in_maps(encoder_outputs, decoder_output, lens)
    res = run_bass_kernel_spmd(
        nc,
        in_maps,
        core_ids=list(range(N_CORES)),
        trace=_trace,
        **(_trace_kwargs or {}),
    )
    context, weights = _combine(res.results)
    if _trace:
        return (context, weights), res
    return context, weights


if __name__ == "__main__":
    rng = np.random.default_rng(0)
    enc = rng.standard_normal((S, B, H), dtype=np.float32)
    dec = rng.standard_normal((T, B, H), dtype=np.float32)
    lens = rng.integers(1, S + 1, size=(B,)).astype(np.int32)
    lens[0] = S
    ctx_out, w_out = kernel(enc, dec, lens)
    print("context", ctx_out.shape, "weights", w_out.shape)
